# revision 10
# baseline (speedup 1.0000x reference)
"""Causal multi-head attention layer (B=2, T=2048, C=2048, H=16) on 8 TRN2
NeuronCores — v6: fp16, per-head software pipeline: the weight-stationary
v-pass for head h+1 is interleaved into the (Act-gated) attention of head h.
(v1 fp32r: 487675ns; v2 bf16: 454541ns; v3: 418466ns; v5: 398823ns.)

Sharding: data-parallel over batch (2 groups of 4 cores), tensor-parallel over
heads within a group (4 heads/core, Megatron column-split of w_attn and
row-split of w_proj).  Each core computes a partial projection output in
transposed layout yT = (O_heads @ w_proj[:, cols].T).T; the host transposes,
sums the 4 partials per batch element and adds b_proj.

v6 structure:
  - everything fp16 (better mantissa than bf16, same PE speed, DVE 2x modes).
  - q/k: weight-stationary u-pairs (8 psum accumulators), first x sweep
    covers the x DMA; x DMA'd as 16 big [128,2048] transfers (HWDGE
    descriptor generation at ~625ns/DMA was gating startup).
  - v: weight-stationary per head (vT form, 16 LDWEIGHTS per head instead of
    64), PE-transposed 128x128-wise into token-major vt.  v(h+1) units are
    interleaved between attention(h) chunks so the PE fills the stalls of
    the Act-bound softmax.
  - attention per head in strip pairs; softmax denominator accumulated on
    DVE (fp16 2x mode) with one ones-matmul per strip; causal diagonal via
    0/1 fp16 mask multiply on DVE; no -1e30 adds, no per-chunk row-sum
    matmuls.
  - PSUM: tag "pa" 2x[128,1024] + "pb" 2x[128,512] + "pc" 2x[128,512]
    (exactly 8 banks), roles rotating per phase.
  - output written as 16 big [128,2048] DMAs.
"""

import numpy as np

import concourse.bacc as bacc
import concourse.tile as tile
from concourse import mybir
from concourse.bass_utils import run_bass_kernel_spmd

F32 = mybir.dt.float32
FP16 = mybir.dt.float16

B, T, C, H = 2, 2048, 2048, 16
HD = C // H            # 128
HLOC = 4               # heads per core
NCORES = 8
NSTRIP = T // 512      # 4 t-strips
NCH = C // 128         # 16 contraction chunks
SCALE = 1.0 / float(np.sqrt(HD))

_cache = {}


def _build_nc(reps=1):
    nc = bacc.Bacc("TRN2", debug=False)

    xt = nc.dram_tensor("xt", [C, T], FP16, kind="ExternalInput")      # x[b].T
    wqkv = nc.dram_tensor("wqkv", [C, 3 * 512], FP16, kind="ExternalInput")
    wp = nc.dram_tensor("wp", [512, C], FP16, kind="ExternalInput")
    mask01_in = nc.dram_tensor("mask01_in", [128, 128], FP16,
                               kind="ExternalInput")
    ones_in = nc.dram_tensor("ones_in", [128, 128], FP16, kind="ExternalInput")
    id_in = nc.dram_tensor("id_in", [128, 128], FP16, kind="ExternalInput")
    yt = nc.dram_tensor("yt", [C, T], FP16, kind="ExternalOutput")

    with tile.TileContext(nc) as tc:
        with (
            tc.tile_pool(name="persist", bufs=1) as persist,
            tc.tile_pool(name="work", bufs=2) as work,
            tc.tile_pool(name="psum", bufs=2, space="PSUM") as psum,
        ):
            qt = persist.tile([128, HLOC * T], FP16, tag="qt")
            kt = persist.tile([128, HLOC * T], FP16, tag="kt")
            vt = persist.tile([128, HLOC * T], FP16, tag="vt")
            ot = persist.tile([128, HLOC * T], FP16, tag="ot")
            mask01 = persist.tile([128, 128], FP16, tag="mask01")
            ones = persist.tile([128, 128], FP16, tag="ones")
            idm = persist.tile([128, 128], FP16, tag="idm")
            nc.sync.dma_start(out=mask01, in_=mask01_in[:, :])
            nc.sync.dma_start(out=ones, in_=ones_in[:, :])
            nc.sync.dma_start(out=idm, in_=id_in[:, :])

            if reps > 1:
                loop_ctx = tc.For_i(
                    0, reps, 1,
                    hint_engines=(mybir.EngineType.PE,
                                  mybir.EngineType.DVE,
                                  mybir.EngineType.Activation,
                                  mybir.EngineType.SP,
                                  mybir.EngineType.Pool))
                loop_ctx.__enter__()

            # ---- DMA: x + w_q interleaved first, then w_k, then w_v ----
            xcs = {}
            wts = {}
            for cc in range(NCH):
                wt = work.tile([128, 512], FP16, tag="wch", bufs=32,
                               name=f"w_0_{cc}")
                nc.sync.dma_start(
                    out=wt, in_=wqkv[128 * cc:128 * (cc + 1), 0:512])
                wts[(0, cc)] = wt
                xcw = work.tile([128, T], FP16, tag="xc", bufs=16,
                                name=f"xc_{cc}")
                nc.sync.dma_start(
                    out=xcw, in_=xt[128 * cc:128 * (cc + 1), :])
                for s in range(NSTRIP):
                    xcs[(cc, s)] = xcw[:, 512 * s:512 * s + 512]
            for pas in (1, 2):
                for cc in range(NCH):
                    wt = work.tile([128, 512], FP16, tag="wch", bufs=32,
                                   name=f"w_{pas}_{cc}")
                    nc.sync.dma_start(
                        out=wt, in_=wqkv[128 * cc:128 * (cc + 1),
                                         512 * pas:512 * (pas + 1)])
                    wts[(pas, cc)] = wt

            def acc8(nm):
                """8 [128,512] psum accumulators: 4 pa-halves + 2 pb + 2 pc."""
                acc2 = [psum.tile([128, 1024], F32, tag="pa", bufs=2,
                                  name=f"{nm}_pa{i}") for i in range(2)]
                out = [acc2[i // 2][:, 512 * (i % 2):512 * (i % 2) + 512]
                       for i in range(4)]
                out += [psum.tile([128, 512], F32, tag="pb", bufs=2,
                                  name=f"{nm}_pb{i}") for i in range(2)]
                out += [psum.tile([128, 512], F32, tag="pc", bufs=2,
                                  name=f"{nm}_pc{i}") for i in range(2)]
                return out

            # ---- q/k: weight-stationary, u-pairs, full-K psum accumulation
            for pas in range(2):            # 0=q, 1=k
                dst = qt if pas == 0 else kt
                for up in range(2):         # u-pair = 2 head-blocks at a time
                    accs = acc8(f"acc_{pas}_{up}")
                    for cc in range(NCH):
                        for u2 in range(2):
                            u = 2 * up + u2
                            w_u = wts[(pas, cc)][:, 128 * u:128 * (u + 1)]
                            for s in range(NSTRIP):
                                nc.tensor.matmul(
                                    accs[4 * u2 + s], lhsT=w_u,
                                    rhs=xcs[(cc, s)],
                                    start=(cc == 0), stop=(cc == NCH - 1))
                    for u2 in range(2):
                        u = 2 * up + u2
                        for s in range(NSTRIP):
                            dslice = dst[:, T * u + 512 * s:
                                         T * u + 512 * (s + 1)]
                            if (u2 + s) % 2 == 0:
                                nc.scalar.copy(dslice, accs[4 * u2 + s])
                            else:
                                nc.vector.tensor_copy(dslice, accs[4 * u2 + s])

            # ---- v (weight-stationary per head u, PE-transposed to vt) ----
            def v_units(u):
                units = []
                halves = [None] * 4
                vsbs = [None] * 4

                def mm_unit(cc):
                    if cc == 0:
                        acc2 = [psum.tile([128, 1024], F32, tag="pa", bufs=2,
                                          name=f"vacc2_{u}_{i}")
                                for i in range(2)]
                        for s in range(4):
                            halves[s] = acc2[s // 2][:, 512 * (s % 2):
                                                     512 * (s % 2) + 512]
                    w_u = wts[(2, cc)][:, 128 * u:128 * (u + 1)]
                    for s in range(4):
                        nc.tensor.matmul(
                            halves[s], lhsT=w_u, rhs=xcs[(cc, s)],
                            start=(cc == 0), stop=(cc == NCH - 1))

                def cp_unit(s):
                    vsbs[s] = work.tile([128, 512], FP16, tag="vtsb", bufs=4,
                                        name=f"vsb_{u}_{s}")
                    nc.vector.tensor_copy(vsbs[s], halves[s])

                def tr_unit(s, tb):
                    j = 4 * s + tb
                    tp = psum.tile([128, 128], FP16, tag="pa", bufs=2,
                                   name=f"vtp_{u}_{j}")
                    nc.tensor.transpose(tp, vsbs[s][:, 128 * tb:128 * (tb + 1)],
                                        idm)
                    nc.scalar.copy(
                        vt[:, 512 * j + 128 * u:512 * j + 128 * (u + 1)], tp)

                for cc in range(NCH):
                    units.append(lambda cc=cc: mm_unit(cc))
                for s in range(4):
                    units.append(lambda s=s: cp_unit(s))
                for s in range(4):
                    for tb in range(4):
                        units.append(lambda s=s, tb=tb: tr_unit(s, tb))
                return units

            # ---- attention units for one head (strip pairs) ----
            def make_pair_units(h, sp):
                sa, sb = 2 * sp, 2 * sp + 1
                nj = 4 * (sb + 1)
                st = {}

                def emit_s(j):
                    smin = j // 4
                    kslice = kt[:, T * h + 128 * j:T * h + 128 * (j + 1)]
                    pts = {}
                    for s in (sa, sb):
                        if j >= 4 * (s + 1):
                            continue
                        o = 128 * (j - 4 * s) if s == smin else 0
                        t0 = 512 * s
                        stp = psum.tile([128, 512], F32, tag="pc", bufs=2,
                                        name=f"stp_{h}_{j}_{s}")
                        nc.tensor.matmul(
                            stp[:, o:], lhsT=kslice,
                            rhs=qt[:, T * h + t0 + o:T * h + t0 + 512],
                            start=True, stop=True)
                        pt = work.tile([128, 512], FP16, tag="pt", bufs=6,
                                       name=f"pt_{h}_{j}_{s}")
                        nc.scalar.activation(
                            pt[:, o:], stp[:, o:],
                            mybir.ActivationFunctionType.Exp,
                            scale=SCALE)
                        if s == smin:
                            nc.vector.tensor_mul(
                                pt[:, o:o + 128], pt[:, o:o + 128], mask01)
                        pts[s] = (pt, o)
                    return pts

                def emit_pv(j, pts):
                    vslice = vt[:, 512 * j + 128 * h:512 * j + 128 * (h + 1)]
                    for s, (pt, o) in pts.items():
                        nc.tensor.matmul(
                            st["otp"][s][:, o:], lhsT=vslice, rhs=pt[:, o:],
                            start=(j == 0), stop=(j == 4 * s + 3))
                    for s, (pt, o) in pts.items():
                        if j == 0:
                            nc.vector.tensor_copy(st["rt"][s], pt)
                        else:
                            nc.vector.tensor_add(
                                st["rt"][s][:, o:], st["rt"][s][:, o:],
                                pt[:, o:])
                    for s, (pt, o) in pts.items():
                        if j != 4 * s + 3:
                            continue
                        sump = psum.tile([128, 512], F32, tag="pc", bufs=2,
                                         name=f"sump_{h}_{s}")
                        nc.tensor.matmul(sump, lhsT=ones, rhs=st["rt"][s],
                                         start=True, stop=True)
                        rin = work.tile([128, 512], F32, tag="rin", bufs=2,
                                        name=f"rin_{h}_{s}")
                        nc.vector.reciprocal(rin, sump)
                        t0 = 512 * s
                        nc.vector.tensor_mul(
                            ot[:, T * h + t0:T * h + t0 + 512],
                            st["otp"][s], rin)

                def unit(j):
                    if j == 0:
                        st["otp"] = {
                            s: psum.tile([128, 512], F32, tag="pb", bufs=2,
                                         name=f"otp_{h}_{s}")
                            for s in (sa, sb)}
                        st["rt"] = {
                            s: work.tile([128, 512], FP16, tag="rt", bufs=4,
                                         name=f"rt_{h}_{s}")
                            for s in (sa, sb)}
                    cur = emit_s(j)
                    if j > 0:
                        emit_pv(j - 1, st["prev"])
                    st["prev"] = cur

                units = [lambda j=j: unit(j) for j in range(nj)]
                units.append(lambda: emit_pv(nj - 1, st["prev"]))
                return units

            def attn_units(h):
                return make_pair_units(h, 0) + make_pair_units(h, 1)

            # ---- proj weights: DMA early (arrive during attention) ----
            wpt = {}
            for hp in range(HLOC):
                wtw = work.tile([128, T], FP16, tag="xc", bufs=16,
                                name=f"wpt_{hp}")
                nc.sync.dma_start(
                    out=wtw, in_=wp[128 * hp:128 * (hp + 1), :])
                for cs in range(4):
                    wpt[(hp, cs)] = wtw[:, 512 * cs:512 * (cs + 1)]

            # ---- attention(h) with v(h+1) interleaved ----
            for f in v_units(0):
                f()
            for h in range(HLOC):
                au = attn_units(h)
                vu = v_units(h + 1) if h < HLOC - 1 else []
                vi = 0
                for i, f in enumerate(au):
                    f()
                    want = (i + 1) * len(vu) // len(au)
                    while vi < want:
                        vu[vi]()
                        vi += 1

            # ---- projection  yT[cout, t] = wp-slices.T x ot-strips ----
            for cb in range(16):            # cout 128-blocks
                cs = cb // 4
                cbo = 128 * (cb % 4)
                if cb % 2 == 0:
                    yp2 = [psum.tile([128, 1024], F32, tag="pa", bufs=2,
                                     name=f"yp2_{cb}_{i}") for i in range(2)]
                    ypps = [yp2[s // 2][:, 512 * (s % 2):512 * (s % 2) + 512]
                            for s in range(NSTRIP)]
                else:
                    ypps = [psum.tile([128, 512], F32,
                                      tag=("pb" if s < 2 else "pc"), bufs=2,
                                      name=f"yp_{cb}_{s}")
                            for s in range(NSTRIP)]
                for hp in range(HLOC):
                    w_cb = wpt[(hp, cs)][:, cbo:cbo + 128]
                    for s in range(NSTRIP):
                        nc.tensor.matmul(
                            ypps[s], lhsT=w_cb,
                            rhs=ot[:, T * hp + 512 * s:T * hp + 512 * (s + 1)],
                            start=(hp == 0), stop=(hp == HLOC - 1))
                ysb = work.tile([128, T], FP16, tag="ysb", bufs=2,
                                name=f"ysb_{cb}")
                for s in range(NSTRIP):
                    if (cb + s) % 2 == 0:
                        nc.vector.tensor_copy(
                            ysb[:, 512 * s:512 * (s + 1)], ypps[s])
                    else:
                        nc.scalar.copy(
                            ysb[:, 512 * s:512 * (s + 1)], ypps[s])
                nc.sync.dma_start(
                    out=yt[128 * cb:128 * (cb + 1), :], in_=ysb)

            if reps > 1:
                loop_ctx.__exit__(None, None, None)

    nc.compile()
    _strip_redundant_ldweights(nc)
    return nc


def _strip_redundant_ldweights(nc):
    """Remove back-to-back InstLdweights that reload the exact weights already
    resident in the PE array (legalization emits one per matmul with no dedup;
    each serialized reload costs ~53-107ns on HW).  Only sync-free loads whose
    (weights AP, perf_mode, tile_position) matches the immediately preceding
    PE weight state are dropped; weight state is conservatively reset at block
    boundaries and on any non-matmul PE instruction."""

    def ap_sig(ap):
        try:
            return ap.to_json()
        except Exception:
            return repr(ap)

    for blk in nc.m.functions[0].blocks:
        cur = None
        keep = []
        changed = False
        for inst in blk.instructions:
            if getattr(inst, "engine", None) != mybir.EngineType.PE:
                keep.append(inst)
                continue
            nm = inst.__class__.__name__
            if nm == "InstLdweights":
                sig = (ap_sig(inst.ins[0]), getattr(inst, "perf_mode", None),
                       getattr(inst, "tile_position", None))
                si = inst.sync_info
                sync_free = not (si and (si.on_wait or si.on_update))
                if sig == cur and sync_free:
                    changed = True
                    continue
                cur = sig
            elif nm != "InstMatmult":
                cur = None
            keep.append(inst)
        if changed:
            blk.instructions = keep


def _host_inputs(x, w_attn, w_proj):
    """Per-core input dicts."""
    x = np.asarray(x, dtype=np.float32)
    w_attn = np.asarray(w_attn, dtype=np.float32)
    w_proj = np.asarray(w_proj, dtype=np.float32)

    p = np.arange(128)[:, None]
    f = np.arange(128)[None, :]
    mask01 = np.where(p <= f, 1.0, 0.0).astype(np.float16)
    ones = np.ones((128, 128), dtype=np.float16)
    idm = np.eye(128, dtype=np.float16)

    in_maps = []
    for core in range(NCORES):
        b, g = divmod(core, 4)
        r0 = 512 * g
        wq = w_attn[r0:r0 + 512, :]            # [512, C]
        wk = w_attn[C + r0:C + r0 + 512, :]
        wv = w_attn[2 * C + r0:2 * C + r0 + 512, :]
        wqkv = np.ascontiguousarray(
            np.concatenate([wq.T, wk.T, wv.T], axis=1)).astype(
                np.float16)                    # [C, 1536]
        wpm = np.ascontiguousarray(w_proj[:, r0:r0 + 512].T).astype(
            np.float16)                        # [512, C]
        in_maps.append({
            "xt": np.ascontiguousarray(x[b].T).astype(np.float16),
            "wqkv": wqkv,
            "wp": wpm,
            "mask01_in": mask01,
            "ones_in": ones,
            "id_in": idm,
        })
    return in_maps


def kernel(x, w_attn, w_proj, b_proj):
    if "nc" not in _cache:
        _cache["nc"] = _build_nc()
    nc = _cache["nc"]

    in_maps = _host_inputs(x, w_attn, w_proj)
    res = run_bass_kernel_spmd(nc, in_maps, core_ids=list(range(NCORES)))
    _cache["last_result"] = res
    if res.exec_time_ns is not None:
        print(f"HW exec time: {res.exec_time_ns} ns")

    b_proj = np.asarray(b_proj, dtype=np.float32)
    out = np.empty((B, T, C), dtype=np.float32)
    for b in range(B):
        acc = res.results[4 * b]["yt"].astype(np.float32)
        for g in range(1, 4):
            acc = acc + res.results[4 * b + g]["yt"].astype(np.float32)
        out[b] = acc.T + b_proj[None, :]
    return out


# revision 14
# speedup vs baseline: 1.0863x; 1.0863x over previous
"""Causal multi-head attention layer (B=2, T=2048, C=2048, H=16) on 8 TRN2
NeuronCores — v6: fp16, per-head software pipeline: the weight-stationary
v-pass for head h+1 is interleaved into the (Act-gated) attention of head h.
(v1 fp32r: 487675ns; v2 bf16: 454541ns; v3: 418466ns; v5: 398823ns.)

Sharding: data-parallel over batch (2 groups of 4 cores), tensor-parallel over
heads within a group (4 heads/core, Megatron column-split of w_attn and
row-split of w_proj).  Each core computes a partial projection output in
transposed layout yT = (O_heads @ w_proj[:, cols].T).T; the host transposes,
sums the 4 partials per batch element and adds b_proj.

v6 structure:
  - everything fp16 (better mantissa than bf16, same PE speed, DVE 2x modes).
  - q/k: weight-stationary u-pairs (8 psum accumulators), first x sweep
    covers the x DMA; x DMA'd as 16 big [128,2048] transfers (HWDGE
    descriptor generation at ~625ns/DMA was gating startup).
  - v: weight-stationary per head (vT form, 16 LDWEIGHTS per head instead of
    64), PE-transposed 128x128-wise into token-major vt.  v(h+1) units are
    interleaved between attention(h) chunks so the PE fills the stalls of
    the Act-bound softmax.
  - attention per head in strip pairs; softmax denominator accumulated on
    DVE (fp16 2x mode) with one ones-matmul per strip; causal diagonal via
    0/1 fp16 mask multiply on DVE; no -1e30 adds, no per-chunk row-sum
    matmuls.
  - PSUM: tag "pa" 2x[128,1024] + "pb" 2x[128,512] + "pc" 2x[128,512]
    (exactly 8 banks), roles rotating per phase.
  - output written as 16 big [128,2048] DMAs.
"""

import numpy as np

import concourse.bacc as bacc
import concourse.tile as tile
from concourse import mybir
from concourse.bass_utils import run_bass_kernel_spmd

F32 = mybir.dt.float32
FP16 = mybir.dt.float16

B, T, C, H = 2, 2048, 2048, 16
HD = C // H            # 128
HLOC = 4               # heads per core
NCORES = 8
NSTRIP = T // 512      # 4 t-strips
NCH = C // 128         # 16 contraction chunks
SCALE = 1.0 / float(np.sqrt(HD))

_cache = {}


def _build_nc(reps=1):
    nc = bacc.Bacc("TRN2", debug=False)

    xt = nc.dram_tensor("xt", [C, T], FP16, kind="ExternalInput")      # x[b].T
    wqkv = nc.dram_tensor("wqkv", [C, 3 * 512], FP16, kind="ExternalInput")
    wp = nc.dram_tensor("wp", [512, C], FP16, kind="ExternalInput")
    mask01_in = nc.dram_tensor("mask01_in", [128, 128], FP16,
                               kind="ExternalInput")
    ones_in = nc.dram_tensor("ones_in", [128, 128], FP16, kind="ExternalInput")
    id_in = nc.dram_tensor("id_in", [128, 128], FP16, kind="ExternalInput")
    yt = nc.dram_tensor("yt", [C, T], FP16, kind="ExternalOutput")

    with tile.TileContext(nc) as tc:
        with (
            tc.tile_pool(name="persist", bufs=1) as persist,
            tc.tile_pool(name="work", bufs=2) as work,
            tc.tile_pool(name="psum", bufs=2, space="PSUM") as psum,
        ):
            qt = persist.tile([128, HLOC * T], FP16, tag="qt")
            kt = persist.tile([128, HLOC * T], FP16, tag="kt")
            vt = persist.tile([128, HLOC * T], FP16, tag="vt")
            ot = persist.tile([128, HLOC * T], FP16, tag="ot")
            mask01 = persist.tile([128, 128], FP16, tag="mask01")
            ones = persist.tile([128, 128], FP16, tag="ones")
            idm = persist.tile([128, 128], FP16, tag="idm")
            nc.sync.dma_start(out=mask01, in_=mask01_in[:, :])
            nc.sync.dma_start(out=ones, in_=ones_in[:, :])
            nc.sync.dma_start(out=idm, in_=id_in[:, :])

            if reps > 1:
                loop_ctx = tc.For_i(
                    0, reps, 1,
                    hint_engines=(mybir.EngineType.PE,
                                  mybir.EngineType.DVE,
                                  mybir.EngineType.Activation,
                                  mybir.EngineType.SP,
                                  mybir.EngineType.Pool))
                loop_ctx.__enter__()

            # ---- DMA: x + w_q interleaved first, then w_k, then w_v ----
            xcs = {}
            wts = {}
            for cc in range(NCH):
                wt = work.tile([128, 512], FP16, tag="wch", bufs=32,
                               name=f"w_0_{cc}")
                nc.sync.dma_start(
                    out=wt, in_=wqkv[128 * cc:128 * (cc + 1), 0:512])
                wts[(0, cc)] = wt
                xcw = work.tile([128, T], FP16, tag="xc", bufs=16,
                                name=f"xc_{cc}")
                nc.sync.dma_start(
                    out=xcw, in_=xt[128 * cc:128 * (cc + 1), :])
                for s in range(NSTRIP):
                    xcs[(cc, s)] = xcw[:, 512 * s:512 * s + 512]
            for pas in (1, 2):
                for cc in range(NCH):
                    wt = work.tile([128, 512], FP16, tag="wch", bufs=32,
                                   name=f"w_{pas}_{cc}")
                    nc.sync.dma_start(
                        out=wt, in_=wqkv[128 * cc:128 * (cc + 1),
                                         512 * pas:512 * (pas + 1)])
                    wts[(pas, cc)] = wt

            def acc8(nm):
                """8 [128,512] psum accumulators: 4 pa-halves + 2 pb + 2 pc."""
                acc2 = [psum.tile([128, 1024], F32, tag="pa", bufs=2,
                                  name=f"{nm}_pa{i}") for i in range(2)]
                out = [acc2[i // 2][:, 512 * (i % 2):512 * (i % 2) + 512]
                       for i in range(4)]
                out += [psum.tile([128, 512], F32, tag="pb", bufs=2,
                                  name=f"{nm}_pb{i}") for i in range(2)]
                out += [psum.tile([128, 512], F32, tag="pc", bufs=2,
                                  name=f"{nm}_pc{i}") for i in range(2)]
                return out

            # ---- q/k: weight-stationary, u-pairs, full-K psum accumulation
            def qk_pair(pas, up):
                dst = qt if pas == 0 else kt
                accs = acc8(f"acc_{pas}_{up}")
                for cc in range(NCH):
                    for u2 in range(2):
                        u = 2 * up + u2
                        w_u = wts[(pas, cc)][:, 128 * u:128 * (u + 1)]
                        for s in range(NSTRIP):
                            nc.tensor.matmul(
                                accs[4 * u2 + s], lhsT=w_u,
                                rhs=xcs[(cc, s)],
                                start=(cc == 0), stop=(cc == NCH - 1))
                for u2 in range(2):
                    u = 2 * up + u2
                    for s in range(NSTRIP):
                        dslice = dst[:, T * u + 512 * s:
                                     T * u + 512 * (s + 1)]
                        if (u2 + s) % 2 == 0:
                            nc.scalar.copy(dslice, accs[4 * u2 + s])
                        else:
                            nc.vector.tensor_copy(dslice, accs[4 * u2 + s])

            def k_single_units(u):
                """k-pass for one head-block using only pb+pc psum (pa left
                free for the interleaved v(0) accumulation)."""
                units = []
                accs = [None] * 4

                def mm_unit(cc):
                    if cc == 0:
                        for s in range(NSTRIP):
                            accs[s] = psum.tile(
                                [128, 512], F32,
                                tag=("pb" if s < 2 else "pc"), bufs=2,
                                name=f"acck_{u}_{s}")
                    w_u = wts[(1, cc)][:, 128 * u:128 * (u + 1)]
                    for s in range(NSTRIP):
                        nc.tensor.matmul(
                            accs[s], lhsT=w_u, rhs=xcs[(cc, s)],
                            start=(cc == 0), stop=(cc == NCH - 1))

                def cp_unit(s):
                    dslice = kt[:, T * u + 512 * s:T * u + 512 * (s + 1)]
                    if s % 2 == 0:
                        nc.scalar.copy(dslice, accs[s])
                    else:
                        nc.vector.tensor_copy(dslice, accs[s])

                for cc in range(NCH):
                    units.append(lambda cc=cc: mm_unit(cc))
                for s in range(NSTRIP):
                    units.append(lambda s=s: cp_unit(s))
                return units

            qk_pair(0, 0)
            qk_pair(0, 1)
            qk_pair(1, 0)

            # ---- v (weight-stationary per head u, PE-transposed to vt) ----
            def v_units(u):
                units = []
                halves = [None] * 4
                vsbs = [None] * 4

                def mm_unit(cc):
                    if cc == 0:
                        acc2 = [psum.tile([128, 1024], F32, tag="pa", bufs=2,
                                          name=f"vacc2_{u}_{i}")
                                for i in range(2)]
                        for s in range(4):
                            halves[s] = acc2[s // 2][:, 512 * (s % 2):
                                                     512 * (s % 2) + 512]
                    w_u = wts[(2, cc)][:, 128 * u:128 * (u + 1)]
                    for s in range(4):
                        nc.tensor.matmul(
                            halves[s], lhsT=w_u, rhs=xcs[(cc, s)],
                            start=(cc == 0), stop=(cc == NCH - 1))

                def cp_unit(s):
                    vsbs[s] = work.tile([128, 512], FP16, tag="vtsb", bufs=4,
                                        name=f"vsb_{u}_{s}")
                    nc.vector.tensor_copy(vsbs[s], halves[s])

                def tr_unit(s, tb):
                    j = 4 * s + tb
                    tp = psum.tile([128, 128], FP16, tag="pa", bufs=2,
                                   name=f"vtp_{u}_{j}")
                    nc.tensor.transpose(tp, vsbs[s][:, 128 * tb:128 * (tb + 1)],
                                        idm)
                    dslice = vt[:, 512 * j + 128 * u:512 * j + 128 * (u + 1)]
                    if j % 2 == 0:
                        nc.scalar.copy(dslice, tp)
                    else:
                        nc.vector.tensor_copy(dslice, tp)

                for cc in range(NCH):
                    units.append(lambda cc=cc: mm_unit(cc))
                for s in range(4):
                    units.append(lambda s=s: cp_unit(s))
                for s in range(4):
                    for tb in range(4):
                        units.append(lambda s=s, tb=tb: tr_unit(s, tb))
                return units

            # ---- attention units for one head (strip pairs) ----
            def make_pair_units(h, sp):
                sa, sb = 2 * sp, 2 * sp + 1
                nj = 4 * (sb + 1)
                st = {}

                def emit_s(j):
                    smin = j // 4
                    kslice = kt[:, T * h + 128 * j:T * h + 128 * (j + 1)]
                    pts = {}
                    for s in (sa, sb):
                        if j >= 4 * (s + 1):
                            continue
                        o = 128 * (j - 4 * s) if s == smin else 0
                        t0 = 512 * s
                        stp = psum.tile([128, 512], F32, tag="pc", bufs=2,
                                        name=f"stp_{h}_{j}_{s}")
                        nc.tensor.matmul(
                            stp[:, o:], lhsT=kslice,
                            rhs=qt[:, T * h + t0 + o:T * h + t0 + 512],
                            start=True, stop=True)
                        pt = work.tile([128, 512], FP16, tag="pt", bufs=8,
                                       name=f"pt_{h}_{j}_{s}")
                        nc.scalar.activation(
                            pt[:, o:], stp[:, o:],
                            mybir.ActivationFunctionType.Exp,
                            scale=SCALE)
                        if s == smin:
                            # diag mask on the otherwise-idle GPSIMD engine
                            nc.gpsimd.tensor_mul(
                                pt[:, o:o + 128], pt[:, o:o + 128], mask01)
                        pts[s] = (pt, o)
                    return pts

                def emit_pv(j, pts):
                    vslice = vt[:, 512 * j + 128 * h:512 * j + 128 * (h + 1)]
                    for s, (pt, o) in pts.items():
                        nc.tensor.matmul(
                            st["otp"][s][:, o:], lhsT=vslice, rhs=pt[:, o:],
                            start=(j == 0), stop=(j == 4 * s + 3))
                    for s, (pt, o) in pts.items():
                        if j == 0:
                            nc.vector.tensor_copy(st["rt"][s], pt)
                        else:
                            nc.vector.tensor_add(
                                st["rt"][s][:, o:], st["rt"][s][:, o:],
                                pt[:, o:])
                    for s, (pt, o) in pts.items():
                        if j != 4 * s + 3:
                            continue
                        sump = psum.tile([128, 512], F32, tag="pc", bufs=2,
                                         name=f"sump_{h}_{s}")
                        nc.tensor.matmul(sump, lhsT=ones, rhs=st["rt"][s],
                                         start=True, stop=True)
                        rin = work.tile([128, 512], F32, tag="rin", bufs=2,
                                        name=f"rin_{h}_{s}")
                        nc.vector.reciprocal(rin, sump)
                        t0 = 512 * s
                        nc.vector.tensor_mul(
                            ot[:, T * h + t0:T * h + t0 + 512],
                            st["otp"][s], rin)

                def unit(j):
                    if j == 0:
                        st["otp"] = {
                            s: psum.tile([128, 512], F32, tag="pb", bufs=2,
                                         name=f"otp_{h}_{s}")
                            for s in (sa, sb)}
                        st["rt"] = {
                            s: work.tile([128, 512], FP16, tag="rt", bufs=4,
                                         name=f"rt_{h}_{s}")
                            for s in (sa, sb)}
                    cur = emit_s(j)
                    if j > 0:
                        emit_pv(j - 1, st["prev"])
                    st["prev"] = cur

                units = [lambda j=j: unit(j) for j in range(nj)]
                units.append(lambda: emit_pv(nj - 1, st["prev"]))
                return units

            def attn_units(h):
                return make_pair_units(h, 0) + make_pair_units(h, 1)

            # ---- proj weights: DMA early (arrive during attention) ----
            wpt = {}
            for hp in range(HLOC):
                wtw = work.tile([128, T], FP16, tag="xc", bufs=16,
                                name=f"wpt_{hp}")
                nc.sync.dma_start(
                    out=wtw, in_=wp[128 * hp:128 * (hp + 1), :])
                for cs in range(4):
                    wpt[(hp, cs)] = wtw[:, 512 * cs:512 * (cs + 1)]

            # ---- k(u2), k(u3) interleaved with v(0) (pa is free) ----
            ku = k_single_units(2) + k_single_units(3)
            vu0 = v_units(0)
            vi = 0
            for i, f in enumerate(ku):
                f()
                want = (i + 1) * len(vu0) // len(ku)
                while vi < want:
                    vu0[vi]()
                    vi += 1

            # ---- attention(h) with v(h+1) interleaved ----
            for h in range(HLOC):
                au = attn_units(h)
                vu = v_units(h + 1) if h < HLOC - 1 else []
                vi = 0
                for i, f in enumerate(au):
                    f()
                    want = (i + 1) * len(vu) // len(au)
                    while vi < want:
                        vu[vi]()
                        vi += 1

            # ---- projection  yT[cout, t] = wp-slices.T x ot-strips ----
            for cb in range(16):            # cout 128-blocks
                cs = cb // 4
                cbo = 128 * (cb % 4)
                if cb % 2 == 0:
                    yp2 = [psum.tile([128, 1024], F32, tag="pa", bufs=2,
                                     name=f"yp2_{cb}_{i}") for i in range(2)]
                    ypps = [yp2[s // 2][:, 512 * (s % 2):512 * (s % 2) + 512]
                            for s in range(NSTRIP)]
                else:
                    ypps = [psum.tile([128, 512], F32,
                                      tag=("pb" if s < 2 else "pc"), bufs=2,
                                      name=f"yp_{cb}_{s}")
                            for s in range(NSTRIP)]
                for hp in range(HLOC):
                    w_cb = wpt[(hp, cs)][:, cbo:cbo + 128]
                    for s in range(NSTRIP):
                        nc.tensor.matmul(
                            ypps[s], lhsT=w_cb,
                            rhs=ot[:, T * hp + 512 * s:T * hp + 512 * (s + 1)],
                            start=(hp == 0), stop=(hp == HLOC - 1))
                ysb = work.tile([128, T], FP16, tag="ysb", bufs=2,
                                name=f"ysb_{cb}")
                for s in range(NSTRIP):
                    if (cb + s) % 2 == 0:
                        nc.vector.tensor_copy(
                            ysb[:, 512 * s:512 * (s + 1)], ypps[s])
                    else:
                        nc.scalar.copy(
                            ysb[:, 512 * s:512 * (s + 1)], ypps[s])
                nc.sync.dma_start(
                    out=yt[128 * cb:128 * (cb + 1), :], in_=ysb)

            if reps > 1:
                loop_ctx.__exit__(None, None, None)

    nc.compile()
    _strip_redundant_ldweights(nc)
    return nc


def _strip_redundant_ldweights(nc):
    """Remove back-to-back InstLdweights that reload the exact weights already
    resident in the PE array (legalization emits one per matmul with no dedup;
    each serialized reload costs ~53-107ns on HW).  Only sync-free loads whose
    (weights AP, perf_mode, tile_position) matches the immediately preceding
    PE weight state are dropped; weight state is conservatively reset at block
    boundaries and on any non-matmul PE instruction."""

    def ap_sig(ap):
        try:
            return ap.to_json()
        except Exception:
            return repr(ap)

    for blk in nc.m.functions[0].blocks:
        cur = None
        keep = []
        changed = False
        for inst in blk.instructions:
            if getattr(inst, "engine", None) != mybir.EngineType.PE:
                keep.append(inst)
                continue
            nm = inst.__class__.__name__
            if nm == "InstLdweights":
                sig = (ap_sig(inst.ins[0]), getattr(inst, "perf_mode", None),
                       getattr(inst, "tile_position", None))
                si = inst.sync_info
                sync_free = not (si and (si.on_wait or si.on_update))
                if sig == cur and sync_free:
                    changed = True
                    continue
                cur = sig
            elif nm != "InstMatmult":
                cur = None
            keep.append(inst)
        if changed:
            blk.instructions = keep


def _host_inputs(x, w_attn, w_proj):
    """Per-core input dicts."""
    x = np.asarray(x, dtype=np.float32)
    w_attn = np.asarray(w_attn, dtype=np.float32)
    w_proj = np.asarray(w_proj, dtype=np.float32)

    p = np.arange(128)[:, None]
    f = np.arange(128)[None, :]
    mask01 = np.where(p <= f, 1.0, 0.0).astype(np.float16)
    ones = np.ones((128, 128), dtype=np.float16)
    idm = np.eye(128, dtype=np.float16)

    in_maps = []
    for core in range(NCORES):
        b, g = divmod(core, 4)
        r0 = 512 * g
        wq = w_attn[r0:r0 + 512, :]            # [512, C]
        wk = w_attn[C + r0:C + r0 + 512, :]
        wv = w_attn[2 * C + r0:2 * C + r0 + 512, :]
        wqkv = np.ascontiguousarray(
            np.concatenate([wq.T, wk.T, wv.T], axis=1)).astype(
                np.float16)                    # [C, 1536]
        wpm = np.ascontiguousarray(w_proj[:, r0:r0 + 512].T).astype(
            np.float16)                        # [512, C]
        in_maps.append({
            "xt": np.ascontiguousarray(x[b].T).astype(np.float16),
            "wqkv": wqkv,
            "wp": wpm,
            "mask01_in": mask01,
            "ones_in": ones,
            "id_in": idm,
        })
    return in_maps


def kernel(x, w_attn, w_proj, b_proj):
    if "nc" not in _cache:
        _cache["nc"] = _build_nc()
    nc = _cache["nc"]

    in_maps = _host_inputs(x, w_attn, w_proj)
    res = run_bass_kernel_spmd(nc, in_maps, core_ids=list(range(NCORES)))
    _cache["last_result"] = res
    if res.exec_time_ns is not None:
        print(f"HW exec time: {res.exec_time_ns} ns")

    b_proj = np.asarray(b_proj, dtype=np.float32)
    out = np.empty((B, T, C), dtype=np.float32)
    for b in range(B):
        acc = res.results[4 * b]["yt"].astype(np.float32)
        for g in range(1, 4):
            acc = acc + res.results[4 * b + g]["yt"].astype(np.float32)
        out[b] = acc.T + b_proj[None, :]
    return out
